# revision 16
# baseline (speedup 1.0000x reference)
"""Trainium2 Bass kernel for nn_MultiHeadAttn (unnormalized spatial attention).

Reference computation (per sample s of B=16):
    X = a[s]               # [C=256, HW=4096]  (H=64 rows of W=64)
    QT = wq @ X + bq       # [C, HW]   (q channels on rows)
    KT = wk @ X + bk
    V  = (wv @ X + bv).T   # [HW, C]   (hw on rows)
    per h: attnT_h = K_h @ Q_h^T        # [W, W]  == (Q_h K_h^T)^T
           attoutT_h = V_h^T @ attnT_h  # [C, W]
    out[s] = a[s] + attoutT (reassembled [C, HW])

Distribution choice: data-parallel over batch B — 2 samples per core on
8 cores (every op is per-sample; attention mixes only within an H row,
so there are no cross-core collectives). Per-core device exec is ~1/8 of
the single-core program; profiled HW exec time (max over cores) drops
accordingly. Weights/biases are replicated (384 KB, ~1 us of DMA).

All matmuls in bf16 (fp32 PSUM accumulation); residual added on device;
output stored bf16 (halves device->host bytes; adds ~1e-3 L2 error
against a 2e-2 gate) and cast back to f32 on the host.
"""

import numpy as np
import ml_dtypes

import concourse.bass as bass
import concourse.mybir as mybir
import concourse.tile as tile
from concourse import bacc
from concourse.bass_utils import run_bass_kernel_spmd

BF16 = mybir.dt.bfloat16
F32 = mybir.dt.float32
AF = mybir.ActivationFunctionType

N_CORES = 8
B, C, H, W = 16, 256, 64, 64
HW = H * W               # 4096
S = B // N_CORES         # samples per core = 2
CC = C // 128            # channel chunks = 2


def build_program(repeat=1):
    # repeat > 1 re-processes the same samples (timing amplification only)
    nc = bacc.Bacc("TRN2", target_bir_lowering=False, debug=False)

    a_in = nc.dram_tensor("a_bf", [S, C, HW], BF16, kind="ExternalInput")
    # packed constants (see _make_in_maps): weights [128, 3*512] bf16 with
    # w_all[p, w*512 + cc*256 + o] = w^T[cc*128 + p, o]; biases [128, 4] f32
    # as columns (bq0, bq1, bk0, bk1); bvb [128, 512] f32 = bv tiled twice.
    w_in = nc.dram_tensor("w_all", [128, 3 * 512], BF16, kind="ExternalInput")
    bqk_in = nc.dram_tensor("bqk", [128, 4], F32, kind="ExternalInput")
    bvb_in = nc.dram_tensor("bvb", [128, 512], F32, kind="ExternalInput")
    out_d = nc.dram_tensor("out", [S, C, HW], BF16, kind="ExternalOutput")

    with tile.TileContext(nc) as tc:
        with (
            tc.tile_pool(name="const", bufs=1) as const_pool,
            tc.tile_pool(name="xb", bufs=4) as xb_pool,
            tc.tile_pool(name="qk", bufs=2) as qk_pool,
            tc.tile_pool(name="vsb", bufs=2) as v_pool,
            tc.tile_pool(name="atsb", bufs=6) as at_pool,
            tc.tile_pool(name="osb", bufs=8) as out_pool,
            tc.tile_pool(name="pp", bufs=4, space=bass.MemorySpace.PSUM) as pp_pool,
            tc.tile_pool(name="atp", bufs=2, space=bass.MemorySpace.PSUM) as atp_pool,
            tc.tile_pool(name="aop", bufs=2, space=bass.MemorySpace.PSUM) as aop_pool,
        ):
            # --- input tiles: rotating pool, 2 samples in flight ---
            # a is pre-cast to bf16 on the host.  One [128, 2*HW] tile per
            # sample (cc-major halves); each input DMA covers BOTH channel
            # chunks of a column range via a 3D access pattern, because the
            # sync queue issues descriptors at ~0.6us each — descriptor
            # count, not bandwidth, gates the start of the pipeline.
            def alloc_xb():
                xbt = xb_pool.tile([128, 2 * HW], BF16, tag="xb", name="xbt",
                                   bufs=2)
                return xbt

            def xb_views(xbt):
                return [xbt[:, cc * HW:(cc + 1) * HW] for cc in range(CC)]

            def emit_load(xbt, sv, col_ranges, eng=None):
                s = sv % S
                eng = eng if eng is not None else nc.sync
                src = a_in[s].rearrange("(cc p) hw -> p cc hw", p=128)
                dst = xbt.rearrange("p (cc hw) -> p cc hw", cc=2)
                for c0, c1 in col_ranges:
                    eng.dma_start(dst[:, :, c0:c1], src[:, :, c0:c1])

            # --- persistent constants ---
            # weight view layout: [128 part = c_in within chunk, cols =
            # cc*256 + c_out] per projection.  Prologue order delivers the
            # first group's exact dependencies first (wq oc0 halves via one
            # strided DMA + the block-0 input columns) so PE start is gated
            # only on ~330KB; the remaining weights and geometrically larger
            # input spans ride behind and land ahead of PE demand.
            w_all_sb = const_pool.tile([128, 3 * 512], BF16, tag="w_all")
            bqk_sb = const_pool.tile([128, 4], F32, tag="bqk")
            bvb_sb = const_pool.tile([128, 512], F32, tag="bvb")
            # Queue split: the sync engine's preamble (sem-range clear +
            # table loads) runs ~2us longer than the compute engines', and
            # every queue issues descriptors at ~0.6us each.  Only SP/ACT/
            # gpsimd can issue DMAs, so the first group's dependencies (wq,
            # block 0, biases) go out on the ACT queue, the remaining
            # weights on the gpsimd queue, and sync carries the bulk input
            # stream (blocks 1-7, prefetch) plus all output stores.
            w_in_q = w_in[:, 0:512].rearrange("p (cc o) -> p cc o", cc=2)
            w_sb_q = w_all_sb[:, 0:512].rearrange("p (cc o) -> p cc o", cc=2)
            xb_cur = alloc_xb()
            nc.scalar.dma_start(w_sb_q[:, :, 0:128], w_in_q[:, :, 0:128])
            emit_load(xb_cur, 0, [(0, 512)], eng=nc.scalar)
            nc.scalar.dma_start(bqk_sb[:], bqk_in[:])
            nc.scalar.dma_start(w_sb_q[:, :, 128:256], w_in_q[:, :, 128:256])
            nc.gpsimd.dma_start(w_all_sb[:, 512:1024], w_in[:, 512:1024])
            nc.gpsimd.dma_start(w_all_sb[:, 1024:1536], w_in[:, 1024:1536])
            nc.gpsimd.dma_start(bvb_sb[:], bvb_in[:])
            emit_load(xb_cur, 0, [(512, 1024), (1024, 2048),
                                  (2048, 3072), (3072, 4096)])
            w_sb = {"wq": w_all_sb[:, 0:512],
                    "wk": w_all_sb[:, 512:1024],
                    "wv": w_all_sb[:, 1024:1536]}
            bq_sb = bqk_sb[:, 0:2]
            bk_sb = bqk_sb[:, 2:4]

            # at4 off-diagonal zeros persist across pool rotations: the at
            # pool rotates round-robin with period AT_BUFS, the ACT diag
            # copies never touch the off-diagonal quadrants, so only the
            # first AT_BUFS attn_block calls need the gpsimd memsets.
            AT_BUFS = 6
            attn_calls = [0]

            for sv in range(S * repeat):
                s = sv % S
                xbt = xb_cur
                xb = xb_views(xbt)
                # prefetch the next sample's input while computing this one
                if sv + 1 < S * repeat:
                    xb_cur = alloc_xb()
                    emit_load(xb_cur, sv + 1, [(0, 2048), (2048, 4096)])

                # --- fused per-column-block pipeline ---
                # For each of 8 column blocks (512 hw positions = 8 h rows):
                # Q/K/V projections for that block, then attention for the
                # same block.  This spreads PE/ACT/DVE work evenly in time.
                qt = [qk_pool.tile([128, HW], BF16, tag=f"qt{oc}", name=f"qt{oc}") for oc in range(CC)]
                kt = [qk_pool.tile([128, HW], BF16, tag=f"kt{oc}", name=f"kt{oc}") for oc in range(CC)]
                v_sb = v_pool.tile([128, 32 * C], BF16, tag="v", name="v")

                def attn_block(g, last=False):
                    # attention for group g (emitted one group behind the
                    # projections so the PE has group g+1's projection
                    # matmuls to run while ACT produces at4 for group g)
                    atps = atp_pool.tile([128, 512], F32, tag="atps", name="atps")
                    for jj4 in range(4):
                        j = 4 * g + jj4
                        for cc in range(CC):
                            nc.tensor.matmul(
                                atps[:, jj4 * 128:(jj4 + 1) * 128],
                                kt[cc][:, j * 128:(j + 1) * 128],
                                qt[cc][:, j * 128:(j + 1) * 128],
                                start=(cc == 0),
                                stop=(cc == 1),
                            )
                    # at4 holds 4 block-diagonal [128, 128] attnT matrices
                    at4 = at_pool.tile([128, 512], BF16, tag="at", name="at")
                    at4r = at4.rearrange("p (j n) -> p j n", j=4)
                    atpsr = atps.rearrange("p (j n) -> p j n", j=4)
                    if attn_calls[0] < AT_BUFS:
                        nc.gpsimd.memset(at4r[0:64, :, 64:128], 0.0)
                        nc.gpsimd.memset(at4r[64:128, :, 0:64], 0.0)
                    attn_calls[0] += 1
                    # both diagonal copies stay on ACT: splitting them across
                    # ACT+DVE was tried and regressed (~30us) — the third
                    # writer on at4 adds more semaphore sync than it saves
                    nc.scalar.activation(at4r[0:64, :, 0:64], atpsr[0:64, :, 0:64],
                                         AF.Copy, bias=0.0)
                    nc.scalar.activation(at4r[64:128, :, 64:128],
                                         atpsr[64:128, :, 64:128],
                                         AF.Copy, bias=0.0)

                    aop = [aop_pool.tile([128, 512], F32, tag="aop", name="aop") for _ in range(CC)]
                    for jj4 in range(4):
                        j = 4 * g + jj4
                        for cc in range(CC):
                            nc.tensor.matmul(
                                aop[cc][:, jj4 * 128:(jj4 + 1) * 128],
                                v_sb[:, j * C + cc * 128: j * C + (cc + 1) * 128],
                                at4[:, jj4 * 128:(jj4 + 1) * 128],
                                start=True,
                                stop=True,
                            )
                    # both cc halves land in one tile so a single 3D-AP DMA
                    # stores the group (descriptor-issue rate on the sync
                    # queue is the scarce resource, not bandwidth); the very
                    # last group stores per-cc so the final drain overlaps
                    # the second residual add
                    osb = out_pool.tile([128, 1024], BF16, tag="osb",
                                        name="osb", bufs=4)
                    for cc in range(CC):
                        nc.vector.tensor_add(
                            osb[:, cc * 512:(cc + 1) * 512], aop[cc][:],
                            xb[cc][:, g * 512:(g + 1) * 512]
                        )
                        if last:
                            nc.sync.dma_start(
                                out_d[s, cc * 128:(cc + 1) * 128,
                                      g * 512:(g + 1) * 512],
                                osb[:, cc * 512:(cc + 1) * 512],
                            )
                    if not last:
                        dsto = out_d[s].rearrange("(cc p) hw -> p cc hw", p=128)
                        nc.sync.dma_start(
                            dsto[:, :, g * 512:(g + 1) * 512],
                            osb.rearrange("p (cc n) -> p cc n", cc=2),
                        )

                for g in range(8):
                    t8 = g
                    # Q/K projections for column block t8
                    for wname, bias_sb, dest in (("wq", bq_sb, qt), ("wk", bk_sb, kt)):
                        for oc in range(CC):
                            ps = pp_pool.tile([128, 512], F32, tag="ps", name="ps")
                            for cc in range(CC):
                                nc.tensor.matmul(
                                    ps[:],
                                    w_sb[wname][:, cc * C + oc * 128: cc * C + oc * 128 + 128],
                                    xb[cc][:, t8 * 512:(t8 + 1) * 512],
                                    start=(cc == 0),
                                    stop=(cc == 1),
                                )
                            nc.scalar.activation(
                                dest[oc][:, t8 * 512:(t8 + 1) * 512],
                                ps[:],
                                AF.Identity,
                                bias=bias_sb[:, oc:oc + 1],
                            )
                    # V projection for hw chunks 4g..4g+3 (jj = 2g, 2g+1)
                    # v_sb[p, j*256 + c] = V[j*128 + p, c]
                    for jj in (2 * g, 2 * g + 1):
                        ps = pp_pool.tile([128, 512], F32, tag="ps", name="ps")
                        for u in range(2):
                            j = 2 * jj + u
                            for cc in range(CC):
                                nc.tensor.matmul(
                                    ps[:, u * C:(u + 1) * C],
                                    xb[cc][:, j * 128:(j + 1) * 128],
                                    w_sb["wv"][:, cc * C:(cc + 1) * C],
                                    start=(cc == 0),
                                    stop=(cc == 1),
                                )
                        nc.vector.tensor_add(
                            v_sb[:, jj * 512:(jj + 1) * 512], ps[:], bvb_sb[:]
                        )

                    # attention lags one group behind the projections
                    if g > 0:
                        attn_block(g - 1)
                attn_block(7, last=(sv == S * repeat - 1))
    nc.compile()
    return nc


_NC_CACHE = None


def _get_program():
    global _NC_CACHE
    if _NC_CACHE is None:
        _NC_CACHE = build_program()
    return _NC_CACHE


def _make_in_maps(a, wq, bq, wk, bk, wv, bv):
    bf = ml_dtypes.bfloat16

    def pack_w(w):
        # w [c_out, c_in] -> SBUF view [128, cc*256 + c_out]
        w_t = np.asarray(w, np.float32).T.astype(bf)          # [c_in, c_out]
        return np.ascontiguousarray(
            w_t.reshape(2, 128, C).transpose(1, 0, 2).reshape(128, 2 * C))

    w_all = np.concatenate([pack_w(wq), pack_w(wk), pack_w(wv)], axis=1)
    bq_f = np.asarray(bq, np.float32)
    bk_f = np.asarray(bk, np.float32)
    bqk = np.ascontiguousarray(
        np.stack([bq_f[:128], bq_f[128:], bk_f[:128], bk_f[128:]], axis=1))
    bvb = np.tile(np.asarray(bv, np.float32).reshape(1, C), (128, 2))
    a4 = np.asarray(a, np.float32).reshape(B, C, HW).astype(bf)
    in_maps = []
    for i in range(N_CORES):
        in_maps.append({
            "a_bf": np.ascontiguousarray(a4[i * S:(i + 1) * S]),
            "w_all": w_all, "bqk": bqk, "bvb": bvb,
        })
    return in_maps


def _bf16_to_f32(x):
    # exact widening cast; ~10x faster than ml_dtypes astype
    return (x.view(np.uint16).astype(np.uint32) << 16).view(np.float32)


def run(a, wq, bq, wk, bk, wv, bv, trace=False, **trace_kw):
    nc = _get_program()
    in_maps = _make_in_maps(a, wq, bq, wk, bk, wv, bv)
    res = run_bass_kernel_spmd(nc, in_maps, list(range(N_CORES)), trace=trace, **trace_kw)
    out = np.concatenate([np.asarray(r["out"]) for r in res.results], axis=0)
    return _bf16_to_f32(np.ascontiguousarray(out)).reshape(B, C, H, W), res


_JIT_CACHE = None


def _get_jit():
    """Cached 8-core shard_map jit of the bass_exec call (avoids the
    re-trace + NEFF-rehash cost run_bass_via_pjrt pays per invocation)."""
    global _JIT_CACHE
    if _JIT_CACHE is None:
        import jax
        from jax.sharding import Mesh, PartitionSpec
        from jax.experimental.shard_map import shard_map
        import concourse.mybir as _mybir
        from concourse import bass2jax

        nc = _get_program()
        bass2jax.install_neuronx_cc_hook()
        partition_name = (nc.partition_id_tensor.name
                          if nc.partition_id_tensor else None)
        in_names, out_names, out_avals, zero_outs = [], [], [], []
        for alloc in nc.m.functions[0].allocations:
            if not isinstance(alloc, _mybir.MemoryLocationSet):
                continue
            name = alloc.memorylocations[0].name
            if alloc.kind == "ExternalInput":
                if name != partition_name:
                    in_names.append(name)
            elif alloc.kind == "ExternalOutput":
                shape = tuple(alloc.tensor_shape)
                dtype = _mybir.dt.np(alloc.dtype)
                out_avals.append(jax.core.ShapedArray(shape, dtype))
                out_names.append(name)
                zero_outs.append(np.zeros(shape, dtype))
        all_in_names = (tuple(in_names) + tuple(out_names)
                        + ((partition_name,) if partition_name else ()))

        def _body(*args):
            operands = list(args)
            if partition_name is not None:
                operands.append(bass2jax.partition_id_tensor())
            return tuple(bass2jax._bass_exec_p.bind(
                *operands,
                out_avals=tuple(out_avals),
                in_names=all_in_names,
                out_names=tuple(out_names),
                lowering_input_output_aliases=(),
                sim_require_finite=True,
                sim_require_nnan=True,
                nc=nc,
            ))

        devices = jax.devices()[:N_CORES]
        mesh = Mesh(np.asarray(devices), ("core",))
        n_in = len(in_names) + len(out_names)
        fn = jax.jit(
            shard_map(_body, mesh=mesh,
                      in_specs=(PartitionSpec("core"),) * n_in,
                      out_specs=(PartitionSpec("core"),) * len(out_names),
                      check_rep=False),
            keep_unused=True,
        )
        concat_zeros = [np.zeros((N_CORES * z.shape[0], *z.shape[1:]), z.dtype)
                        for z in zero_outs]
        _JIT_CACHE = (fn, tuple(in_names), tuple(out_avals), concat_zeros)
    return _JIT_CACHE


def kernel(a, wq, bq, wk, bk, wv, bv):
    fn, in_names, out_avals, concat_zeros = _get_jit()
    in_maps = _make_in_maps(a, wq, bq, wk, bk, wv, bv)
    concat_in = [np.concatenate([m[n] for m in in_maps], axis=0)
                 for n in in_names]
    outs = fn(*concat_in, *concat_zeros)
    out = np.ascontiguousarray(np.asarray(outs[0]))
    return _bf16_to_f32(out).reshape(B, C, H, W)



# revision 23
# speedup vs baseline: 1.0862x; 1.0862x over previous
"""Trainium2 Bass kernel for nn_MultiHeadAttn (unnormalized spatial attention).

Reference computation (per sample s of B=16):
    X = a[s]               # [C=256, HW=4096]  (H=64 rows of W=64)
    QT = wq @ X + bq       # [C, HW]   (q channels on rows)
    KT = wk @ X + bk
    V  = (wv @ X + bv).T   # [HW, C]   (hw on rows)
    per h: attnT_h = K_h @ Q_h^T        # [W, W]  == (Q_h K_h^T)^T
           attoutT_h = V_h^T @ attnT_h  # [C, W]
    out[s] = a[s] + attoutT (reassembled [C, HW])

Distribution choice: data-parallel over batch B — 2 samples per core on
8 cores (every op is per-sample; attention mixes only within an H row,
so there are no cross-core collectives). Per-core device exec is ~1/8 of
the single-core program; profiled HW exec time (max over cores) drops
accordingly. Weights/biases are replicated (384 KB, ~1 us of DMA).

All matmuls in bf16 (fp32 PSUM accumulation); residual added on device;
output stored bf16 (halves device->host bytes; adds ~1e-3 L2 error
against a 2e-2 gate) and cast back to f32 on the host.
"""

import numpy as np
import ml_dtypes

import concourse.bass as bass
import concourse.mybir as mybir
import concourse.tile as tile
from concourse import bacc
from concourse.bass_utils import run_bass_kernel_spmd

BF16 = mybir.dt.bfloat16
F32 = mybir.dt.float32
AF = mybir.ActivationFunctionType

N_CORES = 8
B, C, H, W = 16, 256, 64, 64
HW = H * W               # 4096
S = B // N_CORES         # samples per core = 2
CC = C // 128            # channel chunks = 2


def build_program(repeat=1):
    # repeat > 1 re-processes the same samples (timing amplification only)
    nc = bacc.Bacc("TRN2", target_bir_lowering=False, debug=False)

    a_in = nc.dram_tensor("a_bf", [S, C, HW], BF16, kind="ExternalInput")
    # packed constants (see _make_in_maps): weights [128, 3*512] bf16 with
    # w_all[p, w*512 + cc*256 + o] = w^T[cc*128 + p, o]; biases [128, 4] f32
    # as columns (bq0, bq1, bk0, bk1); bvb [128, 512] f32 = bv tiled twice.
    w_in = nc.dram_tensor("w_all", [128, 3 * 512], BF16, kind="ExternalInput")
    bqk_in = nc.dram_tensor("bqk", [128, 4], F32, kind="ExternalInput")
    bvr_in = nc.dram_tensor("bv_row", [1, 512], BF16, kind="ExternalInput")
    out_d = nc.dram_tensor("out", [S, C, HW], BF16, kind="ExternalOutput")

    with tile.TileContext(nc) as tc:
        with (
            tc.tile_pool(name="const", bufs=1) as const_pool,
            tc.tile_pool(name="xb", bufs=4) as xb_pool,
            tc.tile_pool(name="qk", bufs=2) as qk_pool,
            tc.tile_pool(name="vsb", bufs=2) as v_pool,
            tc.tile_pool(name="atsb", bufs=6) as at_pool,
            tc.tile_pool(name="osb", bufs=8) as out_pool,
            tc.tile_pool(name="pp", bufs=4, space=bass.MemorySpace.PSUM) as pp_pool,
            tc.tile_pool(name="atp", bufs=2, space=bass.MemorySpace.PSUM) as atp_pool,
            tc.tile_pool(name="aop", bufs=2, space=bass.MemorySpace.PSUM) as aop_pool,
        ):
            # --- input tiles: rotating pool, 2 samples in flight ---
            # a is pre-cast to bf16 on the host.  One [128, 2*HW] tile per
            # sample (cc-major halves); each input DMA covers BOTH channel
            # chunks of a column range via a 3D access pattern, because the
            # sync queue issues descriptors at ~0.6us each — descriptor
            # count, not bandwidth, gates the start of the pipeline.
            def alloc_xb():
                xbt = xb_pool.tile([128, 2 * HW], BF16, tag="xb", name="xbt",
                                   bufs=2)
                return xbt

            def xb_views(xbt):
                return [xbt[:, cc * HW:(cc + 1) * HW] for cc in range(CC)]

            def emit_load(xbt, sv, col_ranges, eng=None):
                s = sv % S
                eng = eng if eng is not None else nc.sync
                src = a_in[s].rearrange("(cc p) hw -> p cc hw", p=128)
                dst = xbt.rearrange("p (cc hw) -> p cc hw", cc=2)
                for c0, c1 in col_ranges:
                    eng.dma_start(dst[:, :, c0:c1], src[:, :, c0:c1])

            # --- persistent constants ---
            # weight view layout: [128 part = c_in within chunk, cols =
            # cc*256 + c_out] per projection.  Prologue order delivers the
            # first group's exact dependencies first (wq oc0 halves via one
            # strided DMA + the block-0 input columns) so PE start is gated
            # only on ~330KB; the remaining weights and geometrically larger
            # input spans ride behind and land ahead of PE demand.
            w_all_sb = const_pool.tile([128, 3 * 512], BF16, tag="w_all")
            bqk_sb = const_pool.tile([128, 4], F32, tag="bqk")
            bvb_sb = const_pool.tile([128, 512], F32, tag="bvb")
            # All DMAs ride ONE queue (sync): every queue is gated behind
            # the same all-engine preamble barrier (~7us), descriptor issue
            # costs ~0.65us each, and parallel queues destroy the byte-
            # delivery order (measured: block 0 behind 1.8MB of later
            # blocks, first matmul 16.9us); the gpsimd SWDGE path costs
            # another ~6us of descriptor-build drains.  So: strict priority
            # order on sync, and keep the early stream lean — bvb (bv tiled
            # across partitions, 262KB of redundancy) is built on-device
            # with a K=1 matmul from a 1KB bv row instead of DMAed.
            bvr_sb = const_pool.tile([1, 512], BF16, tag="bvr")
            ones_sb = const_pool.tile([1, 128], BF16, tag="ones")
            xb_cur = alloc_xb()
            nc.sync.dma_start(w_all_sb[:, 0:512], w_in[:, 0:512])
            emit_load(xb_cur, 0, [(0, 512)])
            nc.sync.dma_start(bvr_sb[:], bvr_in[:])
            nc.sync.dma_start(w_all_sb[:, 512:1024], w_in[:, 512:1024])
            nc.sync.dma_start(bqk_sb[:], bqk_in[:])
            nc.sync.dma_start(w_all_sb[:, 1024:1536], w_in[:, 1024:1536])
            emit_load(xb_cur, 0, [(512, 1024), (1024, 2048),
                                  (2048, 3072), (3072, 4096)])
            nc.gpsimd.memset(ones_sb[:], 1.0)
            w_sb = {"wq": w_all_sb[:, 0:512],
                    "wk": w_all_sb[:, 512:1024],
                    "wv": w_all_sb[:, 1024:1536]}
            bq_sb = bqk_sb[:, 0:2]
            bk_sb = bqk_sb[:, 2:4]

            # at4 off-diagonal zeros persist across pool rotations: the at
            # pool rotates round-robin with period AT_BUFS, the ACT diag
            # copies never touch the off-diagonal quadrants, so only the
            # first AT_BUFS attn_block calls need the gpsimd memsets.
            AT_BUFS = 6
            attn_calls = [0]

            for sv in range(S * repeat):
                s = sv % S
                xbt = xb_cur
                xb = xb_views(xbt)
                # prefetch the next sample's input while computing this one
                if sv + 1 < S * repeat:
                    xb_cur = alloc_xb()
                    emit_load(xb_cur, sv + 1, [(0, 2048), (2048, 4096)])

                # --- fused per-column-block pipeline ---
                # For each of 8 column blocks (512 hw positions = 8 h rows):
                # Q/K/V projections for that block, then attention for the
                # same block.  This spreads PE/ACT/DVE work evenly in time.
                qt = [qk_pool.tile([128, HW], BF16, tag=f"qt{oc}", name=f"qt{oc}") for oc in range(CC)]
                kt = [qk_pool.tile([128, HW], BF16, tag=f"kt{oc}", name=f"kt{oc}") for oc in range(CC)]
                v_sb = v_pool.tile([128, 32 * C], BF16, tag="v", name="v")

                def attn_block(g, last=False, halves=1):
                    # attention for group g (emitted one group behind the
                    # projections so the PE has group g+1's projection
                    # matmuls to run while ACT produces at4 for group g).
                    # halves=2 pipelines the atps->at4->aop chain at 2-block
                    # granularity — used at the very end of the program when
                    # no projection work is left to hide the chain latency.
                    atps = atp_pool.tile([128, 512], F32, tag="atps", name="atps")
                    at4 = at_pool.tile([128, 512], BF16, tag="at", name="at")
                    at4r = at4.rearrange("p (j n) -> p j n", j=4)
                    atpsr = atps.rearrange("p (j n) -> p j n", j=4)
                    if attn_calls[0] < AT_BUFS:
                        nc.gpsimd.memset(at4r[0:64, :, 64:128], 0.0)
                        nc.gpsimd.memset(at4r[64:128, :, 0:64], 0.0)
                    attn_calls[0] += 1
                    aop = [aop_pool.tile([128, 512], F32, tag="aop", name="aop") for _ in range(CC)]
                    nh = 4 // halves
                    for h in range(halves):
                        jj4s = range(h * nh, (h + 1) * nh)
                        for jj4 in jj4s:
                            j = 4 * g + jj4
                            for cc in range(CC):
                                nc.tensor.matmul(
                                    atps[:, jj4 * 128:(jj4 + 1) * 128],
                                    kt[cc][:, j * 128:(j + 1) * 128],
                                    qt[cc][:, j * 128:(j + 1) * 128],
                                    start=(cc == 0),
                                    stop=(cc == 1),
                                )
                        # both diagonal copies stay on ACT: splitting them
                        # across ACT+DVE was tried and regressed (~30us) —
                        # the third writer on at4 adds more semaphore sync
                        # than it saves
                        j0, j1 = h * nh, (h + 1) * nh
                        nc.scalar.activation(at4r[0:64, j0:j1, 0:64],
                                             atpsr[0:64, j0:j1, 0:64],
                                             AF.Copy, bias=0.0)
                        nc.scalar.activation(at4r[64:128, j0:j1, 64:128],
                                             atpsr[64:128, j0:j1, 64:128],
                                             AF.Copy, bias=0.0)
                        for jj4 in jj4s:
                            j = 4 * g + jj4
                            for cc in range(CC):
                                nc.tensor.matmul(
                                    aop[cc][:, jj4 * 128:(jj4 + 1) * 128],
                                    v_sb[:, j * C + cc * 128: j * C + (cc + 1) * 128],
                                    at4[:, jj4 * 128:(jj4 + 1) * 128],
                                    start=True,
                                    stop=True,
                                )
                    # both cc halves land in one tile so a single 3D-AP DMA
                    # stores the group (descriptor-issue rate on the sync
                    # queue is the scarce resource, not bandwidth); the very
                    # last group stores per-cc so the final drain overlaps
                    # the second residual add
                    osb = out_pool.tile([128, 1024], BF16, tag="osb",
                                        name="osb", bufs=4)
                    for cc in range(CC):
                        nc.vector.tensor_add(
                            osb[:, cc * 512:(cc + 1) * 512], aop[cc][:],
                            xb[cc][:, g * 512:(g + 1) * 512]
                        )
                        if last:
                            nc.sync.dma_start(
                                out_d[s, cc * 128:(cc + 1) * 128,
                                      g * 512:(g + 1) * 512],
                                osb[:, cc * 512:(cc + 1) * 512],
                            )
                    if not last:
                        dsto = out_d[s].rearrange("(cc p) hw -> p cc hw", p=128)
                        nc.sync.dma_start(
                            dsto[:, :, g * 512:(g + 1) * 512],
                            osb.rearrange("p (cc n) -> p cc n", cc=2),
                        )

                for g in range(8):
                    t8 = g
                    # Q/K projections for column block t8
                    for wname, bias_sb, dest in (("wq", bq_sb, qt), ("wk", bk_sb, kt)):
                        for oc in range(CC):
                            ps = pp_pool.tile([128, 512], F32, tag="ps", name="ps")
                            for cc in range(CC):
                                nc.tensor.matmul(
                                    ps[:],
                                    w_sb[wname][:, cc * C + oc * 128: cc * C + oc * 128 + 128],
                                    xb[cc][:, t8 * 512:(t8 + 1) * 512],
                                    start=(cc == 0),
                                    stop=(cc == 1),
                                )
                            nc.scalar.activation(
                                dest[oc][:, t8 * 512:(t8 + 1) * 512],
                                ps[:],
                                AF.Identity,
                                bias=bias_sb[:, oc:oc + 1],
                            )
                    if sv == 0 and g == 0:
                        # build bvb (= bv broadcast to all 128 partitions)
                        # on-device: a K=1 matmul of ones^T x bv_row.  Emitted
                        # after group-0 Q/K so the PE stream isn't blocked
                        # waiting for the bv_row DMA at program start.
                        ps_bvb = pp_pool.tile([128, 512], F32, tag="ps",
                                              name="ps_bvb")
                        nc.tensor.matmul(ps_bvb[:], ones_sb[:], bvr_sb[:],
                                         start=True, stop=True)
                        nc.scalar.activation(bvb_sb[:], ps_bvb[:], AF.Copy,
                                             bias=0.0)
                    # V projection for hw chunks 4g..4g+3 (jj = 2g, 2g+1)
                    # v_sb[p, j*256 + c] = V[j*128 + p, c]
                    for jj in (2 * g, 2 * g + 1):
                        ps = pp_pool.tile([128, 512], F32, tag="ps", name="ps")
                        for u in range(2):
                            j = 2 * jj + u
                            for cc in range(CC):
                                nc.tensor.matmul(
                                    ps[:, u * C:(u + 1) * C],
                                    xb[cc][:, j * 128:(j + 1) * 128],
                                    w_sb["wv"][:, cc * C:(cc + 1) * C],
                                    start=(cc == 0),
                                    stop=(cc == 1),
                                )
                        nc.vector.tensor_add(
                            v_sb[:, jj * 512:(jj + 1) * 512], ps[:], bvb_sb[:]
                        )

                    # attention lags one group behind the projections
                    if g > 0:
                        final = sv == S * repeat - 1 and g == 7
                        attn_block(g - 1, halves=2 if final else 1)
                attn_block(7, last=(sv == S * repeat - 1),
                           halves=2 if sv == S * repeat - 1 else 1)
    nc.compile()
    return nc


_NC_CACHE = None


def _get_program():
    global _NC_CACHE
    if _NC_CACHE is None:
        _NC_CACHE = build_program()
    return _NC_CACHE


def _make_in_maps(a, wq, bq, wk, bk, wv, bv):
    bf = ml_dtypes.bfloat16

    def pack_w(w):
        # w [c_out, c_in] -> SBUF view [128, cc*256 + c_out]
        w_t = np.asarray(w, np.float32).T.astype(bf)          # [c_in, c_out]
        return np.ascontiguousarray(
            w_t.reshape(2, 128, C).transpose(1, 0, 2).reshape(128, 2 * C))

    w_all = np.concatenate([pack_w(wq), pack_w(wk), pack_w(wv)], axis=1)
    bq_f = np.asarray(bq, np.float32)
    bk_f = np.asarray(bk, np.float32)
    bqk = np.ascontiguousarray(
        np.stack([bq_f[:128], bq_f[128:], bk_f[:128], bk_f[128:]], axis=1))
    bv_row = np.tile(np.asarray(bv, np.float32).astype(bf).reshape(1, C),
                     (1, 2))
    a4 = np.asarray(a, np.float32).reshape(B, C, HW).astype(bf)
    in_maps = []
    for i in range(N_CORES):
        in_maps.append({
            "a_bf": np.ascontiguousarray(a4[i * S:(i + 1) * S]),
            "w_all": w_all, "bqk": bqk, "bv_row": bv_row,
        })
    return in_maps


def _bf16_to_f32(x):
    # exact widening cast; ~10x faster than ml_dtypes astype
    return (x.view(np.uint16).astype(np.uint32) << 16).view(np.float32)


def run(a, wq, bq, wk, bk, wv, bv, trace=False, **trace_kw):
    nc = _get_program()
    in_maps = _make_in_maps(a, wq, bq, wk, bk, wv, bv)
    res = run_bass_kernel_spmd(nc, in_maps, list(range(N_CORES)), trace=trace, **trace_kw)
    out = np.concatenate([np.asarray(r["out"]) for r in res.results], axis=0)
    return _bf16_to_f32(np.ascontiguousarray(out)).reshape(B, C, H, W), res


_JIT_CACHE = None


def _get_jit():
    """Cached 8-core shard_map jit of the bass_exec call (avoids the
    re-trace + NEFF-rehash cost run_bass_via_pjrt pays per invocation)."""
    global _JIT_CACHE
    if _JIT_CACHE is None:
        import jax
        from jax.sharding import Mesh, PartitionSpec
        from jax.experimental.shard_map import shard_map
        import concourse.mybir as _mybir
        from concourse import bass2jax

        nc = _get_program()
        bass2jax.install_neuronx_cc_hook()
        partition_name = (nc.partition_id_tensor.name
                          if nc.partition_id_tensor else None)
        in_names, out_names, out_avals, zero_outs = [], [], [], []
        for alloc in nc.m.functions[0].allocations:
            if not isinstance(alloc, _mybir.MemoryLocationSet):
                continue
            name = alloc.memorylocations[0].name
            if alloc.kind == "ExternalInput":
                if name != partition_name:
                    in_names.append(name)
            elif alloc.kind == "ExternalOutput":
                shape = tuple(alloc.tensor_shape)
                dtype = _mybir.dt.np(alloc.dtype)
                out_avals.append(jax.core.ShapedArray(shape, dtype))
                out_names.append(name)
                zero_outs.append(np.zeros(shape, dtype))
        all_in_names = (tuple(in_names) + tuple(out_names)
                        + ((partition_name,) if partition_name else ()))

        def _body(*args):
            operands = list(args)
            if partition_name is not None:
                operands.append(bass2jax.partition_id_tensor())
            return tuple(bass2jax._bass_exec_p.bind(
                *operands,
                out_avals=tuple(out_avals),
                in_names=all_in_names,
                out_names=tuple(out_names),
                lowering_input_output_aliases=(),
                sim_require_finite=True,
                sim_require_nnan=True,
                nc=nc,
            ))

        devices = jax.devices()[:N_CORES]
        mesh = Mesh(np.asarray(devices), ("core",))
        n_in = len(in_names) + len(out_names)
        fn = jax.jit(
            shard_map(_body, mesh=mesh,
                      in_specs=(PartitionSpec("core"),) * n_in,
                      out_specs=(PartitionSpec("core"),) * len(out_names),
                      check_rep=False),
            keep_unused=True,
        )
        concat_zeros = [np.zeros((N_CORES * z.shape[0], *z.shape[1:]), z.dtype)
                        for z in zero_outs]
        _JIT_CACHE = (fn, tuple(in_names), tuple(out_avals), concat_zeros)
    return _JIT_CACHE


def kernel(a, wq, bq, wk, bk, wv, bv):
    fn, in_names, out_avals, concat_zeros = _get_jit()
    in_maps = _make_in_maps(a, wq, bq, wk, bk, wv, bv)
    concat_in = [np.concatenate([m[n] for m in in_maps], axis=0)
                 for n in in_names]
    outs = fn(*concat_in, *concat_zeros)
    out = np.ascontiguousarray(np.asarray(outs[0]))
    return _bf16_to_f32(out).reshape(B, C, H, W)

